# revision 11
# baseline (speedup 1.0000x reference)
"""BertLexer Trainium2 kernel.

Computes, for full inputs
    word_indices [16,256] int, span_start/span_end [16,256] int,
    W_embed [50002,256] f32, hidden_states [12,16,512,768] f32
the reference
    word_emb = W_embed[word_indices]                                # [B,W,E]
    bert_sub = hidden_states.mean(axis=0)                           # [B,S,H]
    bert_emb[b,w] = mean(bert_sub[b, span_start:span_end])          # [B,W,H]
    out = concat([word_emb, bert_emb], axis=2)                      # [B,W,E+H]

Strategy: data-parallel over the batch dim across 8 NeuronCores (2 batches
per core).  Per core, the 12-layer mean and the ragged span-mean are fused
into one PE matmul chain: build a span-selection matrix
M[w,s] = (start_w <= s < end_w) / (12*len_w) on-chip, transpose it with the
PE, then accumulate  out_bert = sum_l M @ h[l]  into PSUM with fp32r
matmuls.  Word embeddings are row-gathered from DRAM with indirect DMA.
The kernel is HBM-bound on the hidden_states read (~38MB/core).
"""

import sys

import numpy as np

if "/opt/trn_rl_repo" not in sys.path:
    sys.path.insert(0, "/opt/trn_rl_repo")

import concourse.bacc as bacc
import concourse.bass as bass
import concourse.mybir as mybir
import concourse.tile as tile
from concourse.masks import make_identity

B, W, S, H, L, E, V = 16, 256, 512, 768, 12, 256, 50002
NCORES = 8
BPC = B // NCORES  # batches per core
P = 128
WT = W // P  # word-index tiles per batch
ST = S // P  # subword (contraction) tiles per batch
NCHUNKS = [(0, 512), (512, 256)]  # PSUM-bank-sized pieces of H

F32 = mybir.dt.float32
F32R = mybir.dt.float32r
I32 = mybir.dt.int32


def build_program(reps=1):
    nc = bacc.Bacc(
        "TRN2", target_bir_lowering=False, debug=False, num_devices=NCORES
    )
    wi = nc.dram_tensor("word_indices", [BPC, W], I32, kind="ExternalInput").ap()
    ss = nc.dram_tensor("span_start", [BPC, W], I32, kind="ExternalInput").ap()
    se = nc.dram_tensor("span_end", [BPC, W], I32, kind="ExternalInput").ap()
    emb = nc.dram_tensor("W_embed", [V, E], F32, kind="ExternalInput").ap()
    hs = nc.dram_tensor("hidden_states", [L, BPC, S, H], F32, kind="ExternalInput").ap()
    out = nc.dram_tensor("out", [BPC, W, E + H], F32, kind="ExternalOutput").ap()

    with tile.TileContext(nc) as tc:
        with (
            tc.tile_pool(name="const", bufs=1) as const_pool,
            tc.tile_pool(name="idx", bufs=2) as idx_pool,
            tc.tile_pool(name="mask", bufs=2) as mask_pool,
            tc.tile_pool(name="maskT", bufs=2) as maskT_pool,
            tc.tile_pool(name="hbuf", bufs=24) as h_pool,
            tc.tile_pool(name="hsum", bufs=3) as hsum_pool,
            tc.tile_pool(name="obuf", bufs=2) as o_pool,
            tc.tile_pool(name="ptr", bufs=2, space="PSUM") as ptr_pool,
            tc.tile_pool(name="pout", bufs=1, space="PSUM") as pout_pool,
        ):
            identity = const_pool.tile([P, P], F32)
            make_identity(nc, identity)
            iota_i = const_pool.tile([P, S], I32)
            nc.gpsimd.iota(iota_i, pattern=[[1, S]], base=0, channel_multiplier=0)
            iota_f = const_pool.tile([P, S], F32)
            nc.vector.tensor_copy(iota_f, iota_i)

            for rep in range(reps):
              for b in range(BPC):
                # --- span-selection matrix, transposed: [s, w] ---
                maskT_all = maskT_pool.tile([P, ST * W], F32, tag="maskT")
                wi_tiles = []
                for wt in range(WT):
                    wsl = slice(wt * P, (wt + 1) * P)
                    ss_i = idx_pool.tile([P, 1], I32, tag="ss_i")
                    se_i = idx_pool.tile([P, 1], I32, tag="se_i")
                    wi_i = idx_pool.tile([P, 1], I32, tag="wi_i", bufs=2 * WT)
                    nc.sync.dma_start(out=ss_i, in_=ss[b, wsl, None])
                    nc.sync.dma_start(out=se_i, in_=se[b, wsl, None])
                    nc.sync.dma_start(out=wi_i, in_=wi[b, wsl, None])
                    wi_tiles.append(wi_i)
                    ss_f = idx_pool.tile([P, 1], F32, tag="ss_f")
                    se_f = idx_pool.tile([P, 1], F32, tag="se_f")
                    nc.vector.tensor_copy(ss_f, ss_i)
                    nc.vector.tensor_copy(se_f, se_i)
                    len_f = idx_pool.tile([P, 1], F32, tag="len_f")
                    nc.vector.tensor_tensor(
                        len_f, se_f, ss_f, op=mybir.AluOpType.subtract
                    )
                    rlen = idx_pool.tile([P, 1], F32, tag="rlen")
                    nc.vector.reciprocal(rlen, len_f)
                    scale = idx_pool.tile([P, 1], F32, tag="scale")
                    nc.vector.tensor_scalar_mul(scale, rlen, 1.0 / L)
                    # m1 = (iota >= start) * scale ; m2 = (iota < end)
                    m1 = mask_pool.tile([P, S], F32, tag="m1")
                    nc.vector.tensor_scalar(
                        m1,
                        iota_f,
                        scalar1=ss_f[:, :1],
                        scalar2=scale[:, :1],
                        op0=mybir.AluOpType.is_ge,
                        op1=mybir.AluOpType.mult,
                    )
                    m2 = mask_pool.tile([P, S], F32, tag="m2")
                    nc.vector.tensor_scalar(
                        m2,
                        iota_f,
                        scalar1=se_f[:, :1],
                        scalar2=None,
                        op0=mybir.AluOpType.is_lt,
                    )
                    mM = mask_pool.tile([P, S], F32, tag="mM")
                    nc.vector.tensor_tensor(mM, m1, m2, op=mybir.AluOpType.mult)
                    for st in range(ST):
                        ptr = ptr_pool.tile([P, P], F32, space="PSUM", tag="ptr")
                        nc.tensor.transpose(ptr, mM[:, st * P : (st + 1) * P], identity)
                        col = st * W + wt * P
                        nc.vector.tensor_copy(maskT_all[:, col : col + P], ptr)

                # --- fused layer-mean + span-mean:  pout[wt] = sum_l M @ h[l] ---
                pouts = [
                    pout_pool.tile(
                        [P, H], F32, space="PSUM", tag=f"pout{wt}", name=f"pout{wt}_{b}"
                    )
                    for wt in range(WT)
                ]
                for st in range(ST):
                    ssl = slice(st * P, (st + 1) * P)
                    # exact f32 12-layer sum on DVE: hsum = sum_l h[l]
                    hts = []
                    for l in range(L):
                        ht = h_pool.tile([P, H], F32, tag="ht", name=f"ht_{b}_{st}_{l}")
                        nc.sync.dma_start(out=ht, in_=hs[l, b, ssl, :])
                        hts.append(ht)
                    hsum = hsum_pool.tile([P, H], F32, tag="hsum")
                    nc.vector.tensor_tensor(
                        hsum, hts[0], hts[1], op=mybir.AluOpType.add
                    )
                    for l in range(2, L):
                        nc.vector.tensor_tensor(
                            hsum, hsum, hts[l], op=mybir.AluOpType.add
                        )
                    first = st == 0
                    last = st == ST - 1
                    for wt in range(WT):
                        col = st * W + wt * P
                        lh = maskT_all[:, col : col + P]
                        for n0, nl in NCHUNKS:
                            nc.tensor.matmul(
                                pouts[wt][:, n0 : n0 + nl],
                                lhsT=lh,
                                rhs=hsum[:, n0 : n0 + nl],
                                start=first,
                                stop=last,
                            )

                # --- epilogue: word-embedding gather + stores ---
                for wt in range(WT):
                    wsl = slice(wt * P, (wt + 1) * P)
                    bert_t = o_pool.tile([P, H], F32, tag="bert")
                    nc.vector.tensor_copy(bert_t, pouts[wt])
                    wemb_t = o_pool.tile([P, E], F32, tag="wemb")
                    nc.gpsimd.indirect_dma_start(
                        out=wemb_t,
                        out_offset=None,
                        in_=emb[:, :],
                        in_offset=bass.IndirectOffsetOnAxis(
                            ap=wi_tiles[wt][:, :1], axis=0
                        ),
                    )
                    nc.sync.dma_start(out=out[b, wsl, 0:E], in_=wemb_t)
                    nc.sync.dma_start(out=out[b, wsl, E : E + H], in_=bert_t)

    nc.compile()
    return nc


_NC = None


def _get_program():
    global _NC
    if _NC is None:
        _NC = build_program()
    return _NC


def make_in_maps(word_indices, span_start, span_end, W_embed, hidden_states):
    emb = np.ascontiguousarray(W_embed, dtype=np.float32)
    in_maps = []
    for c in range(NCORES):
        bsl = slice(BPC * c, BPC * (c + 1))
        in_maps.append(
            {
                "word_indices": np.ascontiguousarray(
                    word_indices[bsl], dtype=np.int32
                ),
                "span_start": np.ascontiguousarray(span_start[bsl], dtype=np.int32),
                "span_end": np.ascontiguousarray(span_end[bsl], dtype=np.int32),
                "W_embed": emb,
                "hidden_states": np.ascontiguousarray(
                    hidden_states[:, bsl], dtype=np.float32
                ),
            }
        )
    return in_maps


def run(word_indices, span_start, span_end, W_embed, hidden_states, **run_kwargs):
    from concourse.bass_utils import run_bass_kernel_spmd

    nc = _get_program()
    in_maps = make_in_maps(word_indices, span_start, span_end, W_embed, hidden_states)
    res = run_bass_kernel_spmd(nc, in_maps, core_ids=list(range(NCORES)), **run_kwargs)
    out = np.concatenate([res.results[c]["out"] for c in range(NCORES)], axis=0)
    return out, res


def kernel(word_indices, span_start, span_end, W_embed, hidden_states):
    out, _ = run(word_indices, span_start, span_end, W_embed, hidden_states)
    return out


# revision 15
# speedup vs baseline: 1.0517x; 1.0517x over previous
"""BertLexer Trainium2 kernel.

Computes, for full inputs
    word_indices [16,256] int, span_start/span_end [16,256] int,
    W_embed [50002,256] f32, hidden_states [12,16,512,768] f32
the reference
    word_emb = W_embed[word_indices]                                # [B,W,E]
    bert_sub = hidden_states.mean(axis=0)                           # [B,S,H]
    bert_emb[b,w] = mean(bert_sub[b, span_start:span_end])          # [B,W,H]
    out = concat([word_emb, bert_emb], axis=2)                      # [B,W,E+H]

Strategy: data-parallel over the batch dim across 8 NeuronCores (2 batches
per core).  Per core, the 12-layer mean and the ragged span-mean are fused
into one PE matmul chain: build a span-selection matrix
M[w,s] = (start_w <= s < end_w) / (12*len_w) on-chip, transpose it with the
PE, then accumulate  out_bert = sum_l M @ h[l]  into PSUM with fp32r
matmuls.  Word embeddings are row-gathered from DRAM with indirect DMA.
The kernel is HBM-bound on the hidden_states read (~38MB/core).
"""

import sys

import numpy as np

if "/opt/trn_rl_repo" not in sys.path:
    sys.path.insert(0, "/opt/trn_rl_repo")

import concourse.bacc as bacc
import concourse.bass as bass
import concourse.mybir as mybir
import concourse.tile as tile
from concourse.masks import make_identity

B, W, S, H, L, E, V = 16, 256, 512, 768, 12, 256, 50002
NCORES = 8
BPC = B // NCORES  # batches per core
P = 128
WT = W // P  # word-index tiles per batch
ST = S // P  # subword (contraction) tiles per batch
NCHUNKS = [(0, 512), (512, 256)]  # PSUM-bank-sized pieces of H

F32 = mybir.dt.float32
F32R = mybir.dt.float32r
I32 = mybir.dt.int32


def build_program(reps=1):
    nc = bacc.Bacc(
        "TRN2", target_bir_lowering=False, debug=False, num_devices=NCORES
    )
    wi = nc.dram_tensor("word_indices", [BPC, W], I32, kind="ExternalInput").ap()
    ss = nc.dram_tensor("span_start", [BPC, W], I32, kind="ExternalInput").ap()
    se = nc.dram_tensor("span_end", [BPC, W], I32, kind="ExternalInput").ap()
    emb = nc.dram_tensor("W_embed", [V, E], F32, kind="ExternalInput").ap()
    hs = nc.dram_tensor("hidden_states", [L, BPC, S, H], F32, kind="ExternalInput").ap()
    out = nc.dram_tensor("out", [BPC, W, E + H], F32, kind="ExternalOutput").ap()

    with tile.TileContext(nc) as tc:
        with (
            tc.tile_pool(name="const", bufs=1) as const_pool,
            tc.tile_pool(name="idx", bufs=2) as idx_pool,
            tc.tile_pool(name="mask", bufs=2) as mask_pool,
            tc.tile_pool(name="maskT", bufs=2) as maskT_pool,
            tc.tile_pool(name="hbuf", bufs=36) as h_pool,
            tc.tile_pool(name="hsum", bufs=4) as hsum_pool,
            tc.tile_pool(name="obuf", bufs=2) as o_pool,
            tc.tile_pool(name="ptr", bufs=2, space="PSUM") as ptr_pool,
            tc.tile_pool(name="pout", bufs=1, space="PSUM") as pout_pool,
        ):
            identity = const_pool.tile([P, P], F32)
            make_identity(nc, identity)
            iota_i = const_pool.tile([P, S], I32)
            nc.gpsimd.iota(iota_i, pattern=[[1, S]], base=0, channel_multiplier=0)
            iota_f = const_pool.tile([P, S], F32)
            nc.gpsimd.tensor_copy(iota_f, iota_i)

            for rep in range(reps):
              for b in range(BPC):
                # --- span-selection matrix, transposed: [s, w] ---
                maskT_all = maskT_pool.tile([P, ST * W], F32, tag="maskT")
                wi_tiles = []
                for wt in range(WT):
                    wsl = slice(wt * P, (wt + 1) * P)
                    ss_i = idx_pool.tile([P, 1], I32, tag="ss_i")
                    se_i = idx_pool.tile([P, 1], I32, tag="se_i")
                    wi_i = idx_pool.tile([P, 1], I32, tag="wi_i", bufs=2 * WT)
                    nc.sync.dma_start(out=ss_i, in_=ss[b, wsl, None])
                    nc.sync.dma_start(out=se_i, in_=se[b, wsl, None])
                    nc.sync.dma_start(out=wi_i, in_=wi[b, wsl, None])
                    wi_tiles.append(wi_i)
                    ss_f = idx_pool.tile([P, 1], F32, tag="ss_f")
                    se_f = idx_pool.tile([P, 1], F32, tag="se_f")
                    nc.vector.tensor_copy(ss_f, ss_i)
                    nc.vector.tensor_copy(se_f, se_i)
                    len_f = idx_pool.tile([P, 1], F32, tag="len_f")
                    nc.vector.tensor_tensor(
                        len_f, se_f, ss_f, op=mybir.AluOpType.subtract
                    )
                    rlen = idx_pool.tile([P, 1], F32, tag="rlen")
                    nc.vector.reciprocal(rlen, len_f)
                    scale = idx_pool.tile([P, 1], F32, tag="scale")
                    nc.vector.tensor_scalar_mul(scale, rlen, 1.0 / L)
                    # m1 = (iota >= start) * scale ; m2 = (iota < end)
                    m1 = mask_pool.tile([P, S], F32, tag="m1")
                    nc.vector.tensor_scalar(
                        m1,
                        iota_f,
                        scalar1=ss_f[:, :1],
                        scalar2=scale[:, :1],
                        op0=mybir.AluOpType.is_ge,
                        op1=mybir.AluOpType.mult,
                    )
                    m2 = mask_pool.tile([P, S], F32, tag="m2")
                    nc.vector.tensor_scalar(
                        m2,
                        iota_f,
                        scalar1=se_f[:, :1],
                        scalar2=None,
                        op0=mybir.AluOpType.is_lt,
                    )
                    mM = mask_pool.tile([P, S], F32, tag="mM")
                    nc.vector.tensor_tensor(mM, m1, m2, op=mybir.AluOpType.mult)
                    for st in range(ST):
                        ptr = ptr_pool.tile([P, P], F32, space="PSUM", tag="ptr")
                        nc.tensor.transpose(ptr, mM[:, st * P : (st + 1) * P], identity)
                        col = st * W + wt * P
                        nc.scalar.copy(maskT_all[:, col : col + P], ptr)

                # --- fused layer-mean + span-mean:  pout[wt] = sum_l M @ h[l] ---
                pouts = [
                    pout_pool.tile(
                        [P, H], F32, space="PSUM", tag=f"pout{wt}", name=f"pout{wt}_{b}"
                    )
                    for wt in range(WT)
                ]
                for st in range(ST):
                    ssl = slice(st * P, (st + 1) * P)
                    # exact f32 12-layer sum on DVE: hsum = sum_l h[l]
                    hts = []
                    for l in range(L):
                        ht = h_pool.tile([P, H], F32, tag="ht", name=f"ht_{b}_{st}_{l}")
                        nc.sync.dma_start(out=ht, in_=hs[l, b, ssl, :])
                        hts.append(ht)
                    hsum = hsum_pool.tile([P, H], F32, tag="hsum")
                    nc.vector.tensor_tensor(
                        hsum, hts[0], hts[1], op=mybir.AluOpType.add
                    )
                    for l in range(2, L):
                        nc.vector.tensor_tensor(
                            hsum, hsum, hts[l], op=mybir.AluOpType.add
                        )
                    first = st == 0
                    last = st == ST - 1
                    for wt in range(WT):
                        col = st * W + wt * P
                        lh = maskT_all[:, col : col + P]
                        for n0, nl in NCHUNKS:
                            nc.tensor.matmul(
                                pouts[wt][:, n0 : n0 + nl],
                                lhsT=lh,
                                rhs=hsum[:, n0 : n0 + nl],
                                start=first,
                                stop=last,
                            )

                # --- epilogue: word-embedding gather + stores ---
                for wt in range(WT):
                    wsl = slice(wt * P, (wt + 1) * P)
                    bert_t = o_pool.tile([P, H], F32, tag="bert")
                    nc.vector.tensor_copy(bert_t, pouts[wt])
                    wemb_t = o_pool.tile([P, E], F32, tag="wemb")
                    nc.gpsimd.indirect_dma_start(
                        out=wemb_t,
                        out_offset=None,
                        in_=emb[:, :],
                        in_offset=bass.IndirectOffsetOnAxis(
                            ap=wi_tiles[wt][:, :1], axis=0
                        ),
                    )
                    nc.sync.dma_start(out=out[b, wsl, 0:E], in_=wemb_t)
                    nc.sync.dma_start(out=out[b, wsl, E : E + H], in_=bert_t)

    nc.compile()
    return nc


_NC = None


def _get_program():
    global _NC
    if _NC is None:
        _NC = build_program()
    return _NC


def make_in_maps(word_indices, span_start, span_end, W_embed, hidden_states):
    emb = np.ascontiguousarray(W_embed, dtype=np.float32)
    in_maps = []
    for c in range(NCORES):
        bsl = slice(BPC * c, BPC * (c + 1))
        in_maps.append(
            {
                "word_indices": np.ascontiguousarray(
                    word_indices[bsl], dtype=np.int32
                ),
                "span_start": np.ascontiguousarray(span_start[bsl], dtype=np.int32),
                "span_end": np.ascontiguousarray(span_end[bsl], dtype=np.int32),
                "W_embed": emb,
                "hidden_states": np.ascontiguousarray(
                    hidden_states[:, bsl], dtype=np.float32
                ),
            }
        )
    return in_maps


def run(word_indices, span_start, span_end, W_embed, hidden_states, **run_kwargs):
    from concourse.bass_utils import run_bass_kernel_spmd

    nc = _get_program()
    in_maps = make_in_maps(word_indices, span_start, span_end, W_embed, hidden_states)
    res = run_bass_kernel_spmd(nc, in_maps, core_ids=list(range(NCORES)), **run_kwargs)
    out = np.concatenate([res.results[c]["out"] for c in range(NCORES)], axis=0)
    return out, res


def kernel(word_indices, span_start, span_end, W_embed, hidden_states):
    out, _ = run(word_indices, span_start, span_end, W_embed, hidden_states)
    return out
